# revision 15
# baseline (speedup 1.0000x reference)
"""Trainium2 Bass kernel for nn_CGRegressorAdapter (GNN message passing).

Strategy (cone-restricted):
  - The regression head only reads ONE node per graph (last_idx), so each
    layer of the 8-layer GNN stack only needs the node's influence cone:
    V_4={v} at the top, growing by in-neighborhoods down to V_{-1} (~1400
    nodes max) at the embed layer.  Host prep computes nested cone
    orderings (V_{k+1} is a prefix of V_k) and compacted adjacency slices
    M_l = A[V_{l-2}, V_{l-1}] (edge counts, exact in bf16).
  - Data-parallel over B=32 graphs: 8 cores x 4 slots.  Graphs are sorted
    by cone cost; slot j holds ranks [8j, 8j+8) and is padded to that
    quartile's max sizes, so compute tracks the size distribution.
  - All per-slot device input (embed rhs + adjacency slices) ships as ONE
    pre-transposed bf16 blob [128, W] -> one DMA per slot.
  - Per slot: embed (bf16 hi/lo one-hot matmul, f32-exact), 4 base + 4
    adapter GraphConvs.  m = h @ Wnbr and the self path run f32; the
    aggregation matmul runs single-bf16 (m cast to bf16 on DVE) against
    the bf16 count slices — sim'd end-to-end rel err ~1.5e-3 vs 2e-2 gate.
  - Nested prefix ordering makes the self path a plain prefix slice and
    the final extraction column 0.  Small regression head on-chip in f32.
"""
import numpy as np
import ml_dtypes

import concourse.bass as bass
import concourse.mybir as mybir
from concourse import bacc
from concourse.bass import ts
from concourse.bass_utils import run_bass_kernel_spmd
from concourse.tile import TileContext

BF16 = ml_dtypes.bfloat16
F32 = np.float32

B, N, E, H, L, VOCAB = 32, 2048, 8192, 128, 4, 32
N_CORES = 8
NG = B // N_CORES          # graphs (slots) per core
dt = mybir.dt
Alu = mybir.AluOpType
Act = mybir.ActivationFunctionType

# bias column indices in the packed bias tile
BCOL_BASE = 0      # 0..3  base_b
BCOL_ADAPT = 4     # 4..7  adapt_b
BCOL_HB1 = 8
BCOL_HMID = 9      # 9..11
BCOL_HB5 = 12
NBCOL = 16


def _ceil128(x):
    return max(128, (int(x) + 127) // 128 * 128)


def _spans(width, maxw=512):
    out = []
    off = 0
    while off < width:
        w = min(maxw, width - off)
        out.append((off, w))
        off += w
    return out


def _blob_layout(sizes):
    """Free-axis offsets of the per-slot bf16 blob [128, W].
    Sections: erhs [128, Pm1], then M_l as [128, (pin/128)*pout] l=1..5."""
    Pm1, P0, P1, P2, P3 = sizes
    P4 = 128
    dims = [(Pm1, P0), (P0, P1), (P1, P2), (P2, P3), (P3, P4)]
    lay = {"erhs": (0, Pm1)}
    off = Pm1
    for l, (pin, pout) in enumerate(dims):
        w = (pin // 128) * pout
        lay[f"m{l + 1}"] = (off, w)
        off += w
    lay["_total"] = off
    lay["_dims"] = dims
    return lay


def _build_program(slot_sizes, reps=1):
    """slot_sizes: tuple of 4 tuples (Pm1, P0, P1, P2, P3) padded sizes.
    reps>1 repeats the whole body serially (timing: slope removes
    dispatch overhead)."""
    nc = bacc.Bacc("TRN2", target_bir_lowering=False, debug=False,
                   num_devices=N_CORES)
    f32, bf16 = dt.float32, dt.bfloat16
    P4 = 128
    lays = [_blob_layout(s) for s in slot_sizes]

    embw_d = nc.declare_dram_parameter("embed_w2", [128, 2, H], bf16, isOutput=False)
    bws_d = nc.declare_dram_parameter("bwself2", [L, H, H], bf16, isOutput=False)
    bwn_d = nc.declare_dram_parameter("bwnbr2", [L, H, H], bf16, isOutput=False)
    aws_d = nc.declare_dram_parameter("awself2", [L, H, 2, H], bf16, isOutput=False)
    awn_d = nc.declare_dram_parameter("awnbr2", [L, H, 2, H], bf16, isOutput=False)
    hw1_d = nc.declare_dram_parameter("hwa", [H, 2, H], f32, isOutput=False)
    hwm_d = nc.declare_dram_parameter("hwb", [H, H], f32, isOutput=False)
    hw5_d = nc.declare_dram_parameter("hw5", [H, 1], f32, isOutput=False)
    bias_d = nc.declare_dram_parameter("biases", [H, NBCOL], f32, isOutput=False)
    blob_d = [nc.declare_dram_parameter(f"blob{s}", [128, lays[s]["_total"]],
                                        bf16, isOutput=False)
              for s in range(NG)]
    y_d = nc.declare_dram_parameter("y", [1, NG], f32, isOutput=True)

    with TileContext(nc) as tc:
        with (
            tc.tile_pool(name="const", bufs=1) as const,
            tc.tile_pool(name="state", bufs=1) as state,
            tc.tile_pool(name="mp", bufs=6) as mp,
            tc.tile_pool(name="psum_agg", bufs=2, space="PSUM") as psum_agg,
            tc.tile_pool(name="psum_m", bufs=4, space="PSUM") as psum_m,
        ):
            # ---- all input DMAs issued up front (prefetch) ----
            blob_t = [None] * NG

            def load_blobs():
                # small slots first so compute starts after the smallest
                # transfer; erhs section lands first for the embed
                for s in (3, 2, 1, 0):
                    blob_t[s] = state.tile([128, lays[s]["_total"]], bf16,
                                           tag=f"blob{s}", name=f"blob{s}")
                    esz = lays[s]["erhs"][1]
                    nc.sync.dma_start(blob_t[s][:, :esz], blob_d[s][:, :esz])
                    nc.sync.dma_start(blob_t[s][:, esz:], blob_d[s][:, esz:])

            embw = const.tile([128, 2, H], bf16)
            nc.sync.dma_start(embw[:], embw_d[:])
            bias_t = const.tile([H, NBCOL], f32)
            nc.sync.dma_start(bias_t[:], bias_d[:])
            bws_t, bwn_t, aws_t, awn_t = [], [], [], []
            for i in range(L):
                w = const.tile([H, H], bf16, tag=f"bws{i}")
                nc.sync.dma_start(w[:], bws_d[i])
                bws_t.append(w)
                w = const.tile([H, H], bf16, tag=f"bwn{i}")
                nc.sync.dma_start(w[:], bwn_d[i])
                bwn_t.append(w)
                w = const.tile([H, 2, H], bf16, tag=f"aws{i}")
                nc.sync.dma_start(w[:], aws_d[i])
                aws_t.append(w)
                w = const.tile([H, 2, H], bf16, tag=f"awn{i}")
                nc.sync.dma_start(w[:], awn_d[i])
                awn_t.append(w)
            hw1_t = const.tile([H, 2, H], f32)
            nc.sync.dma_start(hw1_t[:], hw1_d[:])
            hwm_t = const.tile([H, H], f32)
            nc.sync.dma_start(hwm_t[:], hwm_d[:])
            hw5_t = const.tile([H, 1], f32)
            nc.sync.dma_start(hw5_t[:], hw5_d[:])

            gbT = state.tile([128, NG], f32, tag="gb")
            gaT = state.tile([128, NG], f32, tag="ga")

            # per-span PSUM agg tiles are fixed [128,512] and reused by tag
            def get_aggs(width):
                return [(psum_agg.tile([128, 512], f32, tag=f"agg{i % 2}",
                                       name=f"agg{i % 2}"), off, w)
                        for i, (off, w) in enumerate(_spans(width))]

            def gconv(blob, moff, nbr_srcs, self_srcs, p_in, p_out, bias_col,
                      out_tile):
                """nbr_srcs: list of (stateT [128,p_in] bf16, [W_hi, W_lo]
                rhs aps).  self_srcs: list of (stateT, [Wself hi/lo lhsT
                aps]).  blob[:, moff+j*p_out :] holds the bf16 count slice
                for chunk j."""
                nchunks = p_in // 128
                aggs = get_aggs(p_out)
                nterm = sum(len(ws) for _, ws in nbr_srcs)

                def emit_m(j):
                    pm = psum_m.tile([128, 128], f32, tag="pm")
                    k = 0
                    for src, ws in nbr_srcs:
                        for w in ws:
                            nc.tensor.matmul(pm[:], src[:, ts(j, 128)], w,
                                             start=(k == 0),
                                             stop=(k == nterm - 1))
                            k += 1
                    mhi = mp.tile([128, 128], bf16, tag="mhi")
                    if j % 2 == 0:
                        nc.vector.tensor_copy(out=mhi[:], in_=pm[:])
                    else:
                        nc.scalar.copy(mhi[:], pm[:])
                    return mhi

                mq = [emit_m(j) for j in range(min(2, nchunks))]
                # self path: hi/lo bf16 weight pair against bf16 state
                k = 0
                for src, ws in self_srcs:
                    for w in ws:
                        for a, off, wd in aggs:
                            nc.tensor.matmul(a[:, :wd], w, src[:, off:off + wd],
                                             start=(k == 0), stop=False)
                        k += 1
                for j in range(nchunks):
                    mhi = mq.pop(0)
                    if j + 2 < nchunks:
                        mq.append(emit_m(j + 2))
                    base = moff + j * p_out
                    for a, off, wd in aggs:
                        nc.tensor.matmul(a[:, :wd], mhi[:],
                                         blob[:, base + off:base + off + wd],
                                         start=False, stop=(j == nchunks - 1))
                for a, off, wd in aggs:
                    nc.scalar.activation(out_tile[:, off:off + wd],
                                         a[:, :wd], Act.Relu,
                                         bias=bias_t[:, bias_col:bias_col + 1])

            def slot_stages(s):
                """Emission closures for one slot: [embed, base1, adapt1,
                base2, ...].  Two slots are interleaved stage-by-stage so
                each layer-boundary ACT wait is hidden under the other
                slot's matmuls."""
                Pm1, P0, P1, P2, P3 = slot_sizes[s]
                lay = lays[s]
                blob = blob_t[s]
                psz = [P0, P1, P2, P3, P4]
                xT = state.tile([128, Pm1], bf16, tag=f"x{s}", name=f"x{s}")
                lat = [xT] + [state.tile([128, psz[k]], bf16, tag=f"lat{k+1}_{s}",
                                         name=f"lat{k+1}_{s}")
                              for k in range(L)]
                currs = [xT] + [state.tile([128, psz[k + 1]], bf16,
                                           tag=f"curr{k+1}_{s}",
                                           name=f"curr{k+1}_{s}")
                                for k in range(L)]
                pins = [Pm1, P0, P1, P2]
                stages = []

                def embed_stage():
                    eoff = lay["erhs"][0]
                    for i_sp, (a, off, wd) in enumerate(get_aggs(Pm1)):
                        nc.tensor.matmul(a[:, :wd], embw[:, 0, :],
                                         blob[:, eoff + off:eoff + off + wd],
                                         start=True, stop=False)
                        nc.tensor.matmul(a[:, :wd], embw[:, 1, :],
                                         blob[:, eoff + off:eoff + off + wd],
                                         start=False, stop=True)
                        if i_sp % 2 == 0:
                            nc.vector.tensor_copy(out=xT[:, off:off + wd],
                                                  in_=a[:, :wd])
                        else:
                            nc.scalar.copy(xT[:, off:off + wd], a[:, :wd])
                stages.append(embed_stage)

                def base_stage(i):
                    def run():
                        gconv(blob, lay[f"m{i+1}"][0],
                              nbr_srcs=[(lat[i], [bwn_t[i][:]])],
                              self_srcs=[(lat[i], [bws_t[i][:]])],
                              p_in=pins[i], p_out=psz[i],
                              bias_col=BCOL_BASE + i, out_tile=lat[i + 1])
                    return run

                def adapt_stage(i):
                    def run():
                        gconv(blob, lay[f"m{i+2}"][0],
                              nbr_srcs=[(lat[i + 1], [awn_t[i][:, 0, :]]),
                                        (currs[i], [awn_t[i][:, 1, :]])],
                              self_srcs=[(lat[i + 1], [aws_t[i][:, 0, :]]),
                                         (currs[i], [aws_t[i][:, 1, :]])],
                              p_in=psz[i], p_out=psz[i + 1],
                              bias_col=BCOL_ADAPT + i, out_tile=currs[i + 1])
                        if i == L - 1:
                            nc.vector.tensor_copy(out=gbT[:, s:s + 1],
                                                  in_=lat[L][:, 0:1])
                            nc.vector.tensor_copy(out=gaT[:, s:s + 1],
                                                  in_=currs[L][:, 0:1])
                    return run

                for i in range(L):
                    stages.append(base_stage(i))
                    stages.append(adapt_stage(i))
                return stages


            # ---- regression head (all slots at once) ----
            def whole_pass():
                load_blobs()
                streams = [slot_stages(ss) for ss in (3, 2, 1, 0)]
                for stage_row in zip(*streams):
                    for st in stage_row:
                        st()
                emit_head()

            def head_mm(lhsT, rhs, bias_col, func):
                pm = psum_m.tile([128, 128], f32, tag="pm")
                nc.tensor.matmul(pm[:, :NG], lhsT, rhs, start=True, stop=True)
                out = state.tile([128, NG], f32, tag="hy")
                nc.scalar.activation(out[:], pm[:, :NG], func,
                                     bias=bias_t[:, bias_col:bias_col + 1])
                return out

            def emit_head():
                # head with relu-free pairs constant-folded on host:
                # y = ((relu(g@Wa+ba))@Wb+bb -> relu) @ hW5 + hb5
                pm = psum_m.tile([128, 128], f32, tag="pm")
                nc.tensor.matmul(pm[:, :NG], hw1_t[:, 0, :], gbT[:],
                                 start=True, stop=False)
                nc.tensor.matmul(pm[:, :NG], hw1_t[:, 1, :], gaT[:],
                                 start=False, stop=True)
                y1 = state.tile([128, NG], f32, tag="hy")
                nc.scalar.activation(y1[:], pm[:, :NG], Act.Relu,
                                     bias=bias_t[:, BCOL_HB1:BCOL_HB1 + 1])
                y2 = head_mm(hwm_t[:], y1[:], BCOL_HMID + 0, Act.Relu)
                pm5 = psum_m.tile([128, 128], f32, tag="pm")
                nc.tensor.matmul(pm5[:1, :NG], hw5_t[:], y2[:],
                                 start=True, stop=True)
                yout = state.tile([1, NG], f32, tag="yout")
                nc.scalar.activation(yout[:], pm5[:1, :NG], Act.Identity,
                                     bias=bias_t[:1, BCOL_HB5:BCOL_HB5 + 1])
                nc.sync.dma_start(y_d[:], yout[:])

            for _rep in range(reps):
                whole_pass()

    nc.compile()
    return nc


_NC_CACHE = {}
_LAST = {}


def _get_program(reps=1):
    key = (_LAST["slot_sizes"], reps)
    if key not in _NC_CACHE:
        _NC_CACHE[key] = _build_program(_LAST["slot_sizes"], reps=reps)
    return _NC_CACHE[key]


def _cones(edge, last_idx):
    """Nested cone ordering per graph.  Returns (order, sizes[n4..nm1])."""
    out = []
    for g in range(B):
        src, dst = edge[g, 0], edge[g, 1]
        order = [int(last_idx[g])]
        inset = np.zeros(N, bool)
        inset[order[0]] = True
        sizes = [1]
        for _ in range(5):
            new = np.unique(src[inset[dst]])
            new = new[~inset[new]]
            order.extend(new.tolist())
            inset[new] = True
            sizes.append(len(order))
        out.append((np.asarray(order), sizes))
    return out


def _split_hilo(a):
    hi = a.astype(BF16)
    lo = (a - hi.astype(F32)).astype(BF16)
    return hi, lo


def _prep_inputs(inputs):
    """Host-side cone construction + sharding.  Returns list of in_maps."""
    inds = np.asarray(inputs["regular_node_inds"]).astype(np.int64)
    shapes = np.asarray(inputs["regular_node_shapes"], dtype=F32)
    edge = np.asarray(inputs["edge_index"]).astype(np.int64)
    last_idx = np.asarray(inputs["last_idx"]).astype(np.int64)

    cones = _cones(edge, last_idx)
    # sort graphs by cost; slot j <- ranks [8j, 8j+8), core c <- rank 8j+c
    cost = np.array([c[1][5] + c[1][4] for c in cones])
    ranks = np.argsort(-cost, kind="stable")
    assign = ranks.reshape(NG, N_CORES)          # [slot, core] -> graph id
    slot_sizes = []
    for s in range(NG):
        gs = assign[s]
        mx = [max(cones[g][1][k] for g in gs) for k in range(6)]
        # sizes[k] = |V_{4-k}|; padded per level (Pm1,P0,P1,P2,P3)
        slot_sizes.append(tuple(_ceil128(mx[5 - l]) for l in range(5)))
    slot_sizes = tuple(slot_sizes)
    _LAST["slot_sizes"] = slot_sizes
    _LAST["assign"] = assign
    lays = [_blob_layout(s) for s in slot_sizes]

    # embed weights, hi/lo bf16 pair (exact): rows 0..31 table, 32..35 and
    # 36..39 shape_w (paired against shapes_hi / shapes_lo blob rows)
    embed_w = np.zeros((128, H), dtype=F32)
    embed_w[:VOCAB] = np.asarray(inputs["embed_table"], dtype=F32)
    embed_w[VOCAB:VOCAB + 4] = np.asarray(inputs["shape_w"], dtype=F32)
    embed_w[VOCAB + 4:VOCAB + 8] = np.asarray(inputs["shape_w"], dtype=F32)
    ehi, elo = _split_hilo(embed_w)
    # the shape_w rows must stay IDENTICAL in both copies within each of
    # hi/lo (they are), pairing: x = oh@(thi+tlo) + (shi+slo)@(swhi+swlo)
    embed_w2 = np.stack([ehi, elo], axis=1)     # [128, 2, H]

    bws2 = np.asarray(inputs["base_Wself"], dtype=F32).astype(BF16)
    bwn2 = np.asarray(inputs["base_Wnbr"], dtype=F32).astype(BF16)
    aws = np.asarray(inputs["adapt_Wself"], dtype=F32).reshape(L, 2, H, H)
    awn = np.asarray(inputs["adapt_Wnbr"], dtype=F32).reshape(L, 2, H, H)
    aws2 = np.ascontiguousarray(aws.transpose(0, 2, 1, 3)).astype(BF16)
    awn2 = np.ascontiguousarray(awn.transpose(0, 2, 1, 3)).astype(BF16)
    hW1 = np.asarray(inputs["hW1"], np.float64)
    hb1 = np.asarray(inputs["hb1"], np.float64)
    hWm = np.asarray(inputs["hWmid"], np.float64)
    hbm = np.asarray(inputs["hbmid"], np.float64)
    Wa = hW1 @ hWm[0]                       # [2H, H]
    ba = hb1 @ hWm[0] + hbm[0]
    Wb = hWm[1] @ hWm[2]                    # [H, H]
    bb = hbm[1] @ hWm[2] + hbm[2]
    hw1 = np.ascontiguousarray(
        Wa.astype(F32).reshape(2, H, H).transpose(1, 0, 2))

    biases = np.zeros((H, NBCOL), dtype=F32)
    biases[:, BCOL_BASE:BCOL_BASE + L] = np.asarray(inputs["base_b"], dtype=F32).T
    biases[:, BCOL_ADAPT:BCOL_ADAPT + L] = np.asarray(inputs["adapt_b"], dtype=F32).T
    biases[:, BCOL_HB1] = ba.astype(F32)
    biases[:, BCOL_HMID] = bb.astype(F32)
    biases[0, BCOL_HB5] = np.asarray(inputs["hb5"], dtype=F32)[0]

    shared = {
        "embed_w2": embed_w2,
        "bwself2": bws2,
        "bwnbr2": bwn2,
        "awself2": aws2,
        "awnbr2": awn2,
        "hwa": hw1,
        "hwb": np.ascontiguousarray(Wb.astype(F32)),
        "hw5": np.asarray(inputs["hW5"], dtype=F32),
        "biases": biases,
    }
    in_maps = [dict(shared) for _ in range(N_CORES)]
    for s in range(NG):
        Pm1, P0, P1, P2, P3 = slot_sizes[s]
        lay = lays[s]
        for c in range(N_CORES):
            g = assign[s, c]
            order, sizes = cones[g]
            n = len(order)
            pos = np.full(N, -1, np.int64)
            pos[order] = np.arange(n)
            src, dst = edge[g, 0], edge[g, 1]
            ps, pd = pos[src], pos[dst]
            blob = np.zeros((128, lay["_total"]), dtype=BF16)
            # erhs: one-hot rows 0..31, shapes hi rows 32..35, lo rows 36..39
            eoff = lay["erhs"][0]
            erhs = np.zeros((128, Pm1), dtype=F32)
            erhs[inds[g][order], np.arange(n)] = 1.0
            shi, slo = _split_hilo(shapes[g][order].T)
            blob[:, eoff:eoff + Pm1] = erhs.astype(BF16)
            blob[VOCAB:VOCAB + 4, eoff:eoff + n] = shi[:, :n]
            blob[VOCAB + 4:VOCAB + 8, eoff:eoff + n] = slo[:, :n]
            for l, (pin, pout) in enumerate(lay["_dims"]):
                ncols = sizes[4 - l]   # |V_{l-1}|
                M = np.zeros((pin, pout), dtype=F32)
                mask = (pd >= 0) & (pd < ncols)
                np.add.at(M, (ps[mask], pd[mask]), 1.0)
                moff = lay[f"m{l + 1}"][0]
                # [pin, pout] -> [128, (pin/128)*pout], chunk-major on free
                Mt = M.astype(BF16).reshape(pin // 128, 128, pout)
                blob[:, moff:moff + (pin // 128) * pout] = (
                    Mt.transpose(1, 0, 2).reshape(128, -1))
            in_maps[c][f"blob{s}"] = blob
    return in_maps


def kernel(**inputs) -> np.ndarray:
    in_maps = _prep_inputs(inputs)
    nc = _get_program()
    assign = _LAST["assign"]
    # first dispatch after a fresh compile has produced garbage before
    # (axon staging race); run twice and keep the steady-state result
    run_bass_kernel_spmd(nc, in_maps, core_ids=list(range(N_CORES)))
    res = run_bass_kernel_spmd(nc, in_maps, core_ids=list(range(N_CORES)))
    out = np.zeros((B, 1), dtype=F32)
    for c in range(N_CORES):
        yc = np.asarray(res.results[c]["y"]).reshape(NG)
        for s in range(NG):
            out[assign[s, c], 0] = yc[s]
    return out


# revision 17
# speedup vs baseline: 1.5485x; 1.5485x over previous
"""Trainium2 Bass kernel for nn_CGRegressorAdapter (GNN message passing).

Strategy (cone-restricted):
  - The regression head only reads ONE node per graph (last_idx), so each
    layer of the 8-layer GNN stack only needs the node's influence cone:
    V_4={v} at the top, growing by in-neighborhoods down to V_{-1} (~1400
    nodes max) at the embed layer.  Host prep computes nested cone
    orderings (V_{k+1} is a prefix of V_k) and compacted adjacency slices
    M_l = A[V_{l-2}, V_{l-1}] (edge counts, exact in bf16).
  - Data-parallel over B=32 graphs: 8 cores x 4 slots.  Graphs are sorted
    by cone cost; slot j holds ranks [8j, 8j+8) and is padded to that
    quartile's max sizes, so compute tracks the size distribution.
  - All per-slot device input (embed rhs + adjacency slices) ships as ONE
    pre-transposed bf16 blob [128, W] -> one DMA per slot.
  - Per slot: embed (bf16 hi/lo one-hot matmul, f32-exact), 4 base + 4
    adapter GraphConvs.  m = h @ Wnbr and the self path run f32; the
    aggregation matmul runs single-bf16 (m cast to bf16 on DVE) against
    the bf16 count slices — sim'd end-to-end rel err ~1.5e-3 vs 2e-2 gate.
  - Nested prefix ordering makes the self path a plain prefix slice and
    the final extraction column 0.  Small regression head on-chip in f32.
"""
import numpy as np
import ml_dtypes

import concourse.bass as bass
import concourse.mybir as mybir
from concourse import bacc
from concourse.bass import ts
from concourse.bass_utils import run_bass_kernel_spmd
from concourse.tile import TileContext

BF16 = ml_dtypes.bfloat16
F32 = np.float32

B, N, E, H, L, VOCAB = 32, 2048, 8192, 128, 4, 32
N_CORES = 8
NG = B // N_CORES          # graphs (slots) per core
dt = mybir.dt
Alu = mybir.AluOpType
Act = mybir.ActivationFunctionType

# bias column indices in the packed bias tile
BCOL_BASE = 0      # 0..3  base_b
BCOL_ADAPT = 4     # 4..7  adapt_b
BCOL_HB1 = 8
BCOL_HMID = 9      # 9..11
BCOL_HB5 = 12
NBCOL = 16


def _ceil128(x):
    return max(128, (int(x) + 127) // 128 * 128)


def _spans(width, maxw=512):
    out = []
    off = 0
    while off < width:
        w = min(maxw, width - off)
        out.append((off, w))
        off += w
    return out


def _blob_layout(sizes):
    """Free-axis offsets of the per-slot bf16 blob [128, W].
    Sections: erhs [128, Pm1], then M_l as [128, (pin/128)*pout] l=1..5."""
    Pm1, P0, P1, P2, P3 = sizes
    P4 = 128
    dims = [(Pm1, P0), (P0, P1), (P1, P2), (P2, P3), (P3, P4)]
    lay = {"erhs": (0, Pm1)}
    off = Pm1
    for l, (pin, pout) in enumerate(dims):
        w = (pin // 128) * pout
        lay[f"m{l + 1}"] = (off, w)
        off += w
    lay["_total"] = off
    lay["_dims"] = dims
    return lay


def _build_program(slot_sizes, reps=1):
    """slot_sizes: tuple of 4 tuples (Pm1, P0, P1, P2, P3) padded sizes.
    reps>1 repeats the whole body serially (timing: slope removes
    dispatch overhead)."""
    nc = bacc.Bacc("TRN2", target_bir_lowering=False, debug=False,
                   num_devices=N_CORES)
    f32, bf16 = dt.float32, dt.bfloat16
    P4 = 128
    lays = [_blob_layout(s) for s in slot_sizes]

    # all weights packed into two tiles: bf16 (embed + GNN) and f32 (head)
    WB = 2 * H + L * 6 * H          # embw hi/lo + per layer bwn,bws,awn2,aws2
    WF = 3 * H + 1 + NBCOL          # hwa(2H) + hwb(H) + hw5(1) + biases
    wb_d = nc.declare_dram_parameter("wpack_bf", [128, WB], bf16, isOutput=False)
    wf_d = nc.declare_dram_parameter("wpack_f32", [128, WF], f32, isOutput=False)
    blob_d = [nc.declare_dram_parameter(f"blob{s}", [128, lays[s]["_total"]],
                                        bf16, isOutput=False)
              for s in range(NG)]
    y_d = nc.declare_dram_parameter("y", [1, NG], f32, isOutput=True)

    with TileContext(nc) as tc:
        with (
            tc.tile_pool(name="const", bufs=1) as const,
            tc.tile_pool(name="state", bufs=1) as state,
            tc.tile_pool(name="mp", bufs=6) as mp,
            tc.tile_pool(name="psum_agg", bufs=2, space="PSUM") as psum_agg,
            tc.tile_pool(name="psum_m", bufs=4, space="PSUM") as psum_m,
        ):
            # ---- all input DMAs issued up front (prefetch) ----
            blob_t = [None] * NG

            def load_blobs():
                # small slots first so compute starts after the smallest
                # transfer; erhs section lands first for the embed
                for s in (3, 2, 1, 0):
                    blob_t[s] = state.tile([128, lays[s]["_total"]], bf16,
                                           tag=f"blob{s}", name=f"blob{s}")
                    esz = lays[s]["erhs"][1]
                    nc.sync.dma_start(blob_t[s][:, :esz], blob_d[s][:, :esz])
                    nc.sync.dma_start(blob_t[s][:, esz:], blob_d[s][:, esz:])

            wb_t = const.tile([128, WB], bf16)
            nc.sync.dma_start(wb_t[:], wb_d[:])
            wf_t = const.tile([128, WF], f32)
            nc.sync.dma_start(wf_t[:], wf_d[:])
            embw_hi = wb_t[:, 0:H]
            embw_lo = wb_t[:, H:2 * H]
            bwn_t, bws_t, awn_t, aws_t = [], [], [], []
            for i in range(L):
                o = 2 * H + i * 6 * H
                bwn_t.append(wb_t[:, o:o + H])
                bws_t.append(wb_t[:, o + H:o + 2 * H])
                awn_t.append((wb_t[:, o + 2 * H:o + 3 * H],
                              wb_t[:, o + 3 * H:o + 4 * H]))
                aws_t.append((wb_t[:, o + 4 * H:o + 5 * H],
                              wb_t[:, o + 5 * H:o + 6 * H]))
            hwa0 = wf_t[:, 0:H]
            hwa1 = wf_t[:, H:2 * H]
            hwb = wf_t[:, 2 * H:3 * H]
            hw5 = wf_t[:, 3 * H:3 * H + 1]
            BOFF = 3 * H + 1

            def bias_ap(col):
                return wf_t[:, BOFF + col:BOFF + col + 1]

            gbT = state.tile([128, NG], f32, tag="gb")
            gaT = state.tile([128, NG], f32, tag="ga")

            # per-span PSUM agg tiles are fixed [128,512] and reused by tag
            def get_aggs(width):
                return [(psum_agg.tile([128, 512], f32, tag=f"agg{i % 2}",
                                       name=f"agg{i % 2}"), off, w)
                        for i, (off, w) in enumerate(_spans(width))]

            def gconv(blob, moff, nbr_srcs, self_srcs, p_in, p_out, bias_col,
                      out_tile):
                """nbr_srcs: list of (stateT [128,p_in] bf16, [W_hi, W_lo]
                rhs aps).  self_srcs: list of (stateT, [Wself hi/lo lhsT
                aps]).  blob[:, moff+j*p_out :] holds the bf16 count slice
                for chunk j."""
                nchunks = p_in // 128
                aggs = get_aggs(p_out)
                nterm = sum(len(ws) for _, ws in nbr_srcs)

                def emit_m(j):
                    pm = psum_m.tile([128, 128], f32, tag="pm")
                    k = 0
                    for src, ws in nbr_srcs:
                        for w in ws:
                            nc.tensor.matmul(pm[:], src[:, ts(j, 128)], w,
                                             start=(k == 0),
                                             stop=(k == nterm - 1))
                            k += 1
                    mhi = mp.tile([128, 128], bf16, tag="mhi")
                    if j % 2 == 0:
                        nc.vector.tensor_copy(out=mhi[:], in_=pm[:])
                    else:
                        nc.scalar.copy(mhi[:], pm[:])
                    return mhi

                mq = [emit_m(j) for j in range(min(2, nchunks))]
                # self path: hi/lo bf16 weight pair against bf16 state
                k = 0
                for src, ws in self_srcs:
                    for w in ws:
                        for a, off, wd in aggs:
                            nc.tensor.matmul(a[:, :wd], w, src[:, off:off + wd],
                                             start=(k == 0), stop=False)
                        k += 1
                for j in range(nchunks):
                    mhi = mq.pop(0)
                    if j + 2 < nchunks:
                        mq.append(emit_m(j + 2))
                    base = moff + j * p_out
                    for a, off, wd in aggs:
                        nc.tensor.matmul(a[:, :wd], mhi[:],
                                         blob[:, base + off:base + off + wd],
                                         start=False, stop=(j == nchunks - 1))
                for a, off, wd in aggs:
                    nc.scalar.activation(out_tile[:, off:off + wd],
                                         a[:, :wd], Act.Relu,
                                         bias=bias_ap(bias_col))

            def slot_stages(s):
                """Emission closures for one slot: [embed, base1, adapt1,
                base2, ...].  Two slots are interleaved stage-by-stage so
                each layer-boundary ACT wait is hidden under the other
                slot's matmuls."""
                Pm1, P0, P1, P2, P3 = slot_sizes[s]
                lay = lays[s]
                blob = blob_t[s]
                psz = [P0, P1, P2, P3, P4]
                xT = state.tile([128, Pm1], bf16, tag=f"x{s}", name=f"x{s}")
                lat = [xT] + [state.tile([128, psz[k]], bf16, tag=f"lat{k+1}_{s}",
                                         name=f"lat{k+1}_{s}")
                              for k in range(L)]
                currs = [xT] + [state.tile([128, psz[k + 1]], bf16,
                                           tag=f"curr{k+1}_{s}",
                                           name=f"curr{k+1}_{s}")
                                for k in range(L)]
                pins = [Pm1, P0, P1, P2]
                stages = []

                def embed_stage():
                    eoff = lay["erhs"][0]
                    for i_sp, (a, off, wd) in enumerate(get_aggs(Pm1)):
                        nc.tensor.matmul(a[:, :wd], embw_hi,
                                         blob[:, eoff + off:eoff + off + wd],
                                         start=True, stop=False)
                        nc.tensor.matmul(a[:, :wd], embw_lo,
                                         blob[:, eoff + off:eoff + off + wd],
                                         start=False, stop=True)
                        if i_sp % 2 == 0:
                            nc.vector.tensor_copy(out=xT[:, off:off + wd],
                                                  in_=a[:, :wd])
                        else:
                            nc.scalar.copy(xT[:, off:off + wd], a[:, :wd])
                stages.append(embed_stage)

                def base_stage(i):
                    def run():
                        gconv(blob, lay[f"m{i+1}"][0],
                              nbr_srcs=[(lat[i], [bwn_t[i]])],
                              self_srcs=[(lat[i], [bws_t[i]])],
                              p_in=pins[i], p_out=psz[i],
                              bias_col=BCOL_BASE + i, out_tile=lat[i + 1])
                    return run

                def adapt_stage(i):
                    def run():
                        gconv(blob, lay[f"m{i+2}"][0],
                              nbr_srcs=[(lat[i + 1], [awn_t[i][0]]),
                                        (currs[i], [awn_t[i][1]])],
                              self_srcs=[(lat[i + 1], [aws_t[i][0]]),
                                         (currs[i], [aws_t[i][1]])],
                              p_in=psz[i], p_out=psz[i + 1],
                              bias_col=BCOL_ADAPT + i, out_tile=currs[i + 1])
                        if i == L - 1:
                            nc.vector.tensor_copy(out=gbT[:, s:s + 1],
                                                  in_=lat[L][:, 0:1])
                            nc.vector.tensor_copy(out=gaT[:, s:s + 1],
                                                  in_=currs[L][:, 0:1])
                    return run

                for i in range(L):
                    stages.append(base_stage(i))
                    stages.append(adapt_stage(i))
                return stages


            # ---- regression head (all slots at once) ----
            def whole_pass():
                load_blobs()
                streams = [slot_stages(ss) for ss in (3, 2, 1, 0)]
                for stage_row in zip(*streams):
                    for st in stage_row:
                        st()
                emit_head()

            def head_mm(lhsT, rhs, bias_col, func):
                pm = psum_m.tile([128, 128], f32, tag="pm")
                nc.tensor.matmul(pm[:, :NG], lhsT, rhs, start=True, stop=True)
                out = state.tile([128, NG], f32, tag="hy")
                nc.scalar.activation(out[:], pm[:, :NG], func,
                                     bias=bias_ap(bias_col))
                return out

            def emit_head():
                # head with relu-free pairs constant-folded on host:
                # y = ((relu(g@Wa+ba))@Wb+bb -> relu) @ hW5 + hb5
                pm = psum_m.tile([128, 128], f32, tag="pm")
                nc.tensor.matmul(pm[:, :NG], hwa0, gbT[:],
                                 start=True, stop=False)
                nc.tensor.matmul(pm[:, :NG], hwa1, gaT[:],
                                 start=False, stop=True)
                y1 = state.tile([128, NG], f32, tag="hy")
                nc.scalar.activation(y1[:], pm[:, :NG], Act.Relu,
                                     bias=bias_ap(BCOL_HB1))
                y2 = head_mm(hwb, y1[:], BCOL_HMID + 0, Act.Relu)
                pm5 = psum_m.tile([128, 128], f32, tag="pm")
                nc.tensor.matmul(pm5[:1, :NG], hw5, y2[:],
                                 start=True, stop=True)
                yout = state.tile([1, NG], f32, tag="yout")
                nc.scalar.activation(yout[:], pm5[:1, :NG], Act.Identity,
                                     bias=bias_ap(BCOL_HB5)[:1])
                nc.sync.dma_start(y_d[:], yout[:])

            for _rep in range(reps):
                whole_pass()

    nc.compile()
    return nc


_NC_CACHE = {}
_LAST = {}


def _get_program(reps=1):
    key = (_LAST["slot_sizes"], reps)
    if key not in _NC_CACHE:
        _NC_CACHE[key] = _build_program(_LAST["slot_sizes"], reps=reps)
    return _NC_CACHE[key]


def _cones(edge, last_idx):
    """Nested cone ordering per graph.  Returns (order, sizes[n4..nm1])."""
    out = []
    for g in range(B):
        src, dst = edge[g, 0], edge[g, 1]
        order = [int(last_idx[g])]
        inset = np.zeros(N, bool)
        inset[order[0]] = True
        sizes = [1]
        for _ in range(5):
            new = np.unique(src[inset[dst]])
            new = new[~inset[new]]
            order.extend(new.tolist())
            inset[new] = True
            sizes.append(len(order))
        out.append((np.asarray(order), sizes))
    return out


def _split_hilo(a):
    hi = a.astype(BF16)
    lo = (a - hi.astype(F32)).astype(BF16)
    return hi, lo


def _prep_inputs(inputs):
    """Host-side cone construction + sharding.  Returns list of in_maps."""
    inds = np.asarray(inputs["regular_node_inds"]).astype(np.int64)
    shapes = np.asarray(inputs["regular_node_shapes"], dtype=F32)
    edge = np.asarray(inputs["edge_index"]).astype(np.int64)
    last_idx = np.asarray(inputs["last_idx"]).astype(np.int64)

    cones = _cones(edge, last_idx)
    # sort graphs by cost; slot j <- ranks [8j, 8j+8), core c <- rank 8j+c
    cost = np.array([c[1][5] + c[1][4] for c in cones])
    ranks = np.argsort(-cost, kind="stable")
    assign = ranks.reshape(NG, N_CORES)          # [slot, core] -> graph id
    slot_sizes = []
    for s in range(NG):
        gs = assign[s]
        mx = [max(cones[g][1][k] for g in gs) for k in range(6)]
        # sizes[k] = |V_{4-k}|; padded per level (Pm1,P0,P1,P2,P3)
        slot_sizes.append(tuple(_ceil128(mx[5 - l]) for l in range(5)))
    slot_sizes = tuple(slot_sizes)
    _LAST["slot_sizes"] = slot_sizes
    _LAST["assign"] = assign
    lays = [_blob_layout(s) for s in slot_sizes]

    # embed weights, hi/lo bf16 pair (exact): rows 0..31 table, 32..35 and
    # 36..39 shape_w (paired against shapes_hi / shapes_lo blob rows)
    embed_w = np.zeros((128, H), dtype=F32)
    embed_w[:VOCAB] = np.asarray(inputs["embed_table"], dtype=F32)
    embed_w[VOCAB:VOCAB + 4] = np.asarray(inputs["shape_w"], dtype=F32)
    embed_w[VOCAB + 4:VOCAB + 8] = np.asarray(inputs["shape_w"], dtype=F32)
    ehi, elo = _split_hilo(embed_w)
    # the shape_w rows must stay IDENTICAL in both copies within each of
    # hi/lo (they are), pairing: x = oh@(thi+tlo) + (shi+slo)@(swhi+swlo)
    embed_w2 = np.stack([ehi, elo], axis=1)     # [128, 2, H]

    bws2 = np.asarray(inputs["base_Wself"], dtype=F32).astype(BF16)
    bwn2 = np.asarray(inputs["base_Wnbr"], dtype=F32).astype(BF16)
    aws = np.asarray(inputs["adapt_Wself"], dtype=F32).reshape(L, 2, H, H)
    awn = np.asarray(inputs["adapt_Wnbr"], dtype=F32).reshape(L, 2, H, H)
    aws2 = np.ascontiguousarray(aws.transpose(0, 2, 1, 3)).astype(BF16)
    awn2 = np.ascontiguousarray(awn.transpose(0, 2, 1, 3)).astype(BF16)
    hW1 = np.asarray(inputs["hW1"], np.float64)
    hb1 = np.asarray(inputs["hb1"], np.float64)
    hWm = np.asarray(inputs["hWmid"], np.float64)
    hbm = np.asarray(inputs["hbmid"], np.float64)
    Wa = hW1 @ hWm[0]                       # [2H, H]
    ba = hb1 @ hWm[0] + hbm[0]
    Wb = hWm[1] @ hWm[2]                    # [H, H]
    bb = hbm[1] @ hWm[2] + hbm[2]
    hw1 = np.ascontiguousarray(
        Wa.astype(F32).reshape(2, H, H).transpose(1, 0, 2))

    biases = np.zeros((H, NBCOL), dtype=F32)
    biases[:, BCOL_BASE:BCOL_BASE + L] = np.asarray(inputs["base_b"], dtype=F32).T
    biases[:, BCOL_ADAPT:BCOL_ADAPT + L] = np.asarray(inputs["adapt_b"], dtype=F32).T
    biases[:, BCOL_HB1] = ba.astype(F32)
    biases[:, BCOL_HMID] = bb.astype(F32)
    biases[0, BCOL_HB5] = np.asarray(inputs["hb5"], dtype=F32)[0]

    WB = 2 * H + L * 6 * H
    WF = 3 * H + 1 + NBCOL
    wpack_bf = np.zeros((128, WB), dtype=BF16)
    wpack_bf[:, 0:H] = embed_w2[:, 0, :]
    wpack_bf[:, H:2 * H] = embed_w2[:, 1, :]
    for i in range(L):
        o = 2 * H + i * 6 * H
        wpack_bf[:, o:o + H] = bwn2[i]
        wpack_bf[:, o + H:o + 2 * H] = bws2[i]
        wpack_bf[:, o + 2 * H:o + 3 * H] = awn2[i][:, 0, :]
        wpack_bf[:, o + 3 * H:o + 4 * H] = awn2[i][:, 1, :]
        wpack_bf[:, o + 4 * H:o + 5 * H] = aws2[i][:, 0, :]
        wpack_bf[:, o + 5 * H:o + 6 * H] = aws2[i][:, 1, :]
    wpack_f32 = np.zeros((128, WF), dtype=F32)
    wpack_f32[:, 0:H] = Wa.astype(F32)[:H, :]
    wpack_f32[:, H:2 * H] = Wa.astype(F32)[H:, :]
    wpack_f32[:, 2 * H:3 * H] = Wb.astype(F32)
    wpack_f32[:, 3 * H:3 * H + 1] = np.asarray(inputs["hW5"], dtype=F32)
    wpack_f32[:, 3 * H + 1:] = biases
    shared = {"wpack_bf": wpack_bf, "wpack_f32": wpack_f32}
    in_maps = [dict(shared) for _ in range(N_CORES)]
    for s in range(NG):
        Pm1, P0, P1, P2, P3 = slot_sizes[s]
        lay = lays[s]
        for c in range(N_CORES):
            g = assign[s, c]
            order, sizes = cones[g]
            n = len(order)
            pos = np.full(N, -1, np.int64)
            pos[order] = np.arange(n)
            src, dst = edge[g, 0], edge[g, 1]
            ps, pd = pos[src], pos[dst]
            blob = np.zeros((128, lay["_total"]), dtype=BF16)
            # erhs: one-hot rows 0..31, shapes hi rows 32..35, lo rows 36..39
            eoff = lay["erhs"][0]
            erhs = np.zeros((128, Pm1), dtype=F32)
            erhs[inds[g][order], np.arange(n)] = 1.0
            shi, slo = _split_hilo(shapes[g][order].T)
            blob[:, eoff:eoff + Pm1] = erhs.astype(BF16)
            blob[VOCAB:VOCAB + 4, eoff:eoff + n] = shi[:, :n]
            blob[VOCAB + 4:VOCAB + 8, eoff:eoff + n] = slo[:, :n]
            for l, (pin, pout) in enumerate(lay["_dims"]):
                ncols = sizes[4 - l]   # |V_{l-1}|
                M = np.zeros((pin, pout), dtype=F32)
                mask = (pd >= 0) & (pd < ncols)
                np.add.at(M, (ps[mask], pd[mask]), 1.0)
                moff = lay[f"m{l + 1}"][0]
                # [pin, pout] -> [128, (pin/128)*pout], chunk-major on free
                Mt = M.astype(BF16).reshape(pin // 128, 128, pout)
                blob[:, moff:moff + (pin // 128) * pout] = (
                    Mt.transpose(1, 0, 2).reshape(128, -1))
            in_maps[c][f"blob{s}"] = blob
    return in_maps


def kernel(**inputs) -> np.ndarray:
    in_maps = _prep_inputs(inputs)
    nc = _get_program()
    assign = _LAST["assign"]
    # first dispatch after a fresh compile has produced garbage before
    # (axon staging race); run twice and keep the steady-state result
    run_bass_kernel_spmd(nc, in_maps, core_ids=list(range(N_CORES)))
    res = run_bass_kernel_spmd(nc, in_maps, core_ids=list(range(N_CORES)))
    out = np.zeros((B, 1), dtype=F32)
    for c in range(N_CORES):
        yc = np.asarray(res.results[c]["y"]).reshape(NG)
        for s in range(NG):
            out[assign[s, c], 0] = yc[s]
    return out
